# revision 2
# baseline (speedup 1.0000x reference)
"""Bayesian linear layer (reparameterized sample + predictive uncertainty)
as an 8-core SPMD Trainium2 Bass kernel.

Reference computation (all fp32):
    W     = weight_mu + exp(weight_log_sigma) * eps_w          # [OUT, IN]
    b     = bias_mu + exp(bias_log_sigma) * eps_b              # [OUT]
    out   = x @ W.T + b                                        # [B, OUT]
    unc   = sqrt((x*x) @ (exp(weight_log_sigma)**2).T + exp(bias_log_sigma)**2)

Strategy (v4):
  * Sharding: 4 batch-quarters x 2 out-feature-halves = 8 cores.
  * The host only re-lays-out inputs: transpose to contraction-major,
    downcast to bf16 (eps to fp8e4m3 - it enters W scaled by sigma~0.05,
    so fp8 noise is ~0.3% of W), tile into k-major blocks, and slice
    the shards. All arithmetic runs on device.
  * Matmul in bf16 (fp32 PSUM): out^T tiles = W-block.T @ x^T, weights
    stationary, x streaming, no device transposes. (fp8 matmuls fail
    the 2e-2 gate: measured absmax/scale ~3.5e-2.)
  * Input DMA: mu on the sync HWDGE ring; eps+x interleaved per-k on the
    scalar HWDGE ring, with 1-k-tile first chunks so the first sampled
    weight k-tile exists ~2us after the rings start moving. Weight
    sampling (DVE) follows chunk arrival order.
  * Phase 1 uses ALL 8 PSUM banks (o-tiles 0-3; o3's two banks live in
    the aux pool). Warm-up matmuls accumulate exact zeros (ones @ zeros)
    into o3's banks, which phase 1 then extends (start=False) - they are
    read by the flush, so DCE keeps them, and they cover first-DMA
    latency + the HAM clock ramp. Phase 2 = o-tiles 4-6 + the rowsum
    matmuls (aux banks recycled after o3's flush); phase 3 = o-tile 7.
  * Fast path (constant weight_log_sigma): uncertainty collapses to
    sqrt(sigma^2 * rowsum(x^2) + bsig^2). rowsum(x^2) is computed on
    the PE with an ALL-ONES 128x128 stationary - every output partition
    receives the same column sum pre-broadcast in PSUM - and the
    uncertainty is a single Act pass (scale=sigma^2, per-partition
    bias=bsig^2, Sqrt) straight out of PSUM per o-tile. x^2 runs on the
    Act engine into rotating pair buffers feeding a DVE pairwise
    reduction tree (no full x^2 tensor in SBUF).
  * Outputs: o^T flushes (DVE bias-add out of PSUM) DMA per 512-col
    block on the sync ring; u^T tiles DMA on the gpsimd (SWDGE) ring so
    they never queue behind the x/eps input stream.
  * Host transposes and upcasts the o^T/u^T bf16 shards while
    assembling the full [B, OUT] fp32 arrays.
"""

import numpy as np
import ml_dtypes

B, IN, OUT = 4096, 2048, 2048
R, C = 4, 2              # batch split x out-feature split
N_CORES = R * C
BS = B // R              # 1024 batch rows per core
OS = OUT // C            # 1024 out features per core
KT = IN // 128           # 16 contraction k-tiles
OT = OS // 128           # 8 out-feature partition tiles per core
BB = BS // 512           # 2 psum column blocks
NWARM = 8

BF16 = ml_dtypes.bfloat16
FP8 = ml_dtypes.float8_e4m3

TRACE = False            # test harness sets True to capture an NTFF profile
LAST_RESULT = None       # BassKernelResults of the most recent run

_compiled = {}           # cache: key -> compiled Bass program

# k-tile chunk sizes for the input streams / sampling (boundaries of the
# sampling chunks must align with both mu and eps chunk boundaries)
MUCH = [1, 1, 2, 4, 4, 4]
XCH = [1, 1, 2, 4, 4, 4]
ECH = [1, 1, 2, 4, 4, 4]
SCH = [1, 1, 2, 4, 4, 4]


def _build(sigma_const):
    """Build + compile the per-core program. sigma_const=None -> general
    path (log_sigma streamed, second matmul for variance); float -> fast
    path with sigma baked in."""
    import concourse.mybir as mybir
    import concourse.tile as tile
    from concourse import bacc

    F32 = mybir.dt.float32
    BF = mybir.dt.bfloat16
    F8 = mybir.dt.float8e4
    AF = mybir.ActivationFunctionType
    ALU = mybir.AluOpType
    fast = sigma_const is not None

    nc = bacc.Bacc("TRN2", target_bir_lowering=False, debug=False,
                   num_devices=N_CORES)

    # weight-ish tensors are k-tile-major on the free axis; weight blocks
    # additionally o-tile-major: free col = (k*OT + o)*128 + c
    x_d = nc.dram_tensor("x_sh", [128, KT * BS], BF, kind="ExternalInput").ap()
    mu_d = nc.dram_tensor("mu_sh", [128, KT * OS], BF,
                          kind="ExternalInput").ap()
    eps_d = nc.dram_tensor("eps_sh", [128, KT * OS], F8 if fast else BF,
                           kind="ExternalInput").ap()
    if not fast:
        ls_d = nc.dram_tensor("ls_sh", [128, KT * OS], BF,
                              kind="ExternalInput").ap()
    bmu_d = nc.dram_tensor("bmu_sh", [128, OT], F32, kind="ExternalInput").ap()
    bls_d = nc.dram_tensor("bls_sh", [128, OT], F32, kind="ExternalInput").ap()
    beps_d = nc.dram_tensor("beps_sh", [128, OT], F32,
                            kind="ExternalInput").ap()
    o_d = nc.dram_tensor("o_sh", [OS, BS], BF, kind="ExternalOutput").ap()
    u_d = nc.dram_tensor("u_sh", [OS, BS], BF, kind="ExternalOutput").ap()

    with tile.TileContext(nc) as tc:
        with (
            tc.tile_pool(name="big", bufs=1) as big,
            tc.tile_pool(name="stage", bufs=2) as stage,
            tc.tile_pool(name="outs", bufs=3) as outs,
            tc.tile_pool(name="pmain", bufs=6, space="PSUM") as pmain,
            tc.tile_pool(name="paux", bufs=2, space="PSUM") as paux,
        ):
            # ---- resident SBUF tensors ----------------------------------
            if not fast:
                xT = big.tile([128, KT * BS], BF, tag="xT")
                x2T = big.tile([128, KT * BS], BF, tag="x2T")
            wT = big.tile([128, KT * OS], BF, tag="wT")
            if not fast:
                s2T = big.tile([128, KT * OS], BF, tag="s2T")

            ones128 = big.tile([128, 128], BF, tag="ones128")
            rjunk = big.tile([128, 512], BF, tag="rjunk")
            nc.vector.memset(rjunk[:], 0.0)
            nc.vector.memset(ones128[:], 1.0)

            # ---- bias vectors as [128, OT] column grids -----------------
            bmu_sb = big.tile([128, OT], F32, tag="bmu")
            bls_sb = big.tile([128, OT], F32, tag="bls")
            beps_sb = big.tile([128, OT], F32, tag="beps")
            beng = nc.gpsimd if fast else nc.sync
            beng.dma_start(bmu_sb[:], bmu_d[:])
            beng.dma_start(bls_sb[:], bls_d[:])
            beng.dma_start(beps_sb[:], beps_d[:])
            bsig = big.tile([128, OT], F32, tag="bsig")
            bse = big.tile([128, OT], F32, tag="bse")
            bias_all = big.tile([128, OT], F32, tag="bias_all")
            bs2_all = big.tile([128, OT], F32, tag="bs2_all")

            if fast:
                # ---- psum layout -----------------------------------------
                # phase 1: o0-2 in pmain (6 banks) + o3 in paux (2 banks)
                # phase 2: o4-6 in pmain + rs in paux; phase 3: o7 in pmain
                pos = {}
                for o in range(3):
                    for bb in range(BB):
                        pos[(o, bb)] = pmain.tile([128, 512], F32,
                                                  tag="po", name="po")
                for bb in range(BB):
                    pos[(3, bb)] = paux.tile([128, 512], F32,
                                             tag="pa", name="po3")

                # warm-up matmuls: accumulate exact zeros (ones @ zeros)
                # into o3's banks; phase 1's o3 matmuls extend the group
                # (start=False) and the flush reads it, so DCE keeps them.
                # They cover first-DMA latency + the HAM clock ramp.
                for i in range(NWARM):
                    nc.tensor.matmul(pos[(3, i % BB)][:], ones128[:],
                                     rjunk[:], start=(i < BB), stop=False,
                                     skip_group_check=True)

                # ---- input DMA streams ----------------------------------
                # sync ring: mu chunks (k-tile-ramped)
                mu_ts = []
                k0 = 0
                for ci, cn in enumerate(MUCH):
                    t = big.tile([128, cn * OS], BF, tag=f"mu{ci}",
                                 name=f"mu{ci}")
                    nc.sync.dma_start(t[:], mu_d[:, k0 * OS:(k0 + cn) * OS])
                    mu_ts.append((k0, cn, t))
                    k0 += cn

                # scalar ring: eps+x interleaved per chunk so sampling and
                # the rhs stream advance together
                eps_ts = []
                x_ts = []
                ek = 0
                xk = 0
                for ci in range(len(XCH)):
                    en = ECH[ci]
                    et = big.tile([128, en * OS], F8, tag=f"eps{ci}",
                                  name=f"eps{ci}")
                    nc.scalar.dma_start(et[:], eps_d[:, ek * OS:(ek + en) * OS])
                    eps_ts.append((ek, en, et))
                    ek += en
                    cn = XCH[ci]
                    xt = big.tile([128, cn * BS], BF, tag=f"x{ci}",
                                  name=f"x{ci}")
                    nc.scalar.dma_start(xt[:], x_d[:, xk * BS:(xk + cn) * BS])
                    x_ts.append((xk, cn, xt))
                    xk += cn

                def _find(ts, k, n, width):
                    for k0g, cn, t in ts:
                        if k0g <= k and k + n <= k0g + cn:
                            return t[:, (k - k0g) * width:
                                     (k - k0g + n) * width]
                    raise AssertionError

                def xk_slice(k, bb, width=512):
                    for k0g, cn, t in x_ts:
                        if k0g <= k < k0g + cn:
                            c = (k - k0g) * BS + bb * 512
                            return t[:, c:c + width]
                    raise AssertionError

                # ---- weight sampling on DVE, chunk-arrival order --------
                k0 = 0
                for ci, cn in enumerate(SCH):
                    se_t = stage.tile([128, 4 * OS], BF, tag="se", bufs=2)
                    nc.vector.tensor_scalar_mul(se_t[:, :cn * OS],
                                                _find(eps_ts, k0, cn, OS),
                                                float(sigma_const))
                    nc.vector.tensor_tensor(wT[:, k0 * OS:(k0 + cn) * OS],
                                            _find(mu_ts, k0, cn, OS),
                                            se_t[:, :cn * OS], ALU.add)
                    k0 += cn

                # ---- x^2 (Act) + pairwise DVE reduction tree ------------
                # Act squares into rotating pair buffers; DVE adds pairs
                # into L1, then reduces L1->L2->L3->xsum.
                L1 = []
                for p in range(KT // 2):
                    t0 = stage.tile([128, BS], BF, tag="x2", bufs=4)
                    t1 = stage.tile([128, BS], BF, tag="x2", bufs=4)
                    nc.scalar.activation(t0[:], xk_slice(2 * p, 0, BS),
                                         AF.Square)
                    nc.scalar.activation(t1[:], xk_slice(2 * p + 1, 0, BS),
                                         AF.Square)
                    l1 = big.tile([128, BS], BF, tag=f"L1_{p}",
                                  name=f"L1_{p}")
                    nc.vector.tensor_tensor(l1[:], t0[:], t1[:], ALU.add)
                    L1.append(l1)
                lvl = [t[:] for t in L1]
                li = 2
                while len(lvl) > 1:
                    nxt = []
                    for p in range(0, len(lvl), 2):
                        if len(lvl) == 2:
                            dst = big.tile([128, BS], BF, tag="xsum")
                        else:
                            dst = big.tile([128, BS], BF, tag=f"L{li}_{p}",
                                           name=f"L{li}_{p}")
                        nc.vector.tensor_tensor(dst[:], lvl[p], lvl[p + 1],
                                                ALU.add)
                        nxt.append(dst[:])
                    lvl = nxt
                    li += 1
                xsum = lvl[0]

                # ---- bias math (DVE/Act; ready long before first flush) -
                nc.scalar.activation(bsig[:], bls_sb[:], AF.Exp)
                nc.vector.tensor_tensor(bse[:], bsig[:], beps_sb[:],
                                        ALU.mult)
                nc.vector.tensor_tensor(bias_all[:], bmu_sb[:], bse[:],
                                        ALU.add)
                nc.vector.tensor_tensor(bs2_all[:], bsig[:], bsig[:],
                                        ALU.mult)

                def w_blk(k, o):
                    c = (k * OT + o) * 128
                    return wT[:, c:c + 128]

                def flush(o):
                    ot = outs.tile([128, BS], BF, tag="o", bufs=4, name="ot")
                    for bb in range(BB):
                        bsl = slice(bb * 512, (bb + 1) * 512)
                        nc.vector.tensor_scalar_add(ot[:, bsl],
                                                    pos[(o, bb)][:],
                                                    bias_all[:, o:o + 1])
                        nc.sync.dma_start(o_d[o * 128:(o + 1) * 128, bsl],
                                          ot[:, bsl])

                # ---- phase 1: o-tiles 0-3, all 8 banks ------------------
                for k in range(KT):
                    for o in range(4):
                        for bb in range(BB):
                            nc.tensor.matmul(pos[(o, bb)][:], w_blk(k, o),
                                             xk_slice(k, bb),
                                             start=(k == 0 and o < 3),
                                             stop=(k == KT - 1),
                                             skip_group_check=(o == 3))
                for o in range(4):
                    flush(o)

                # ---- phase 2: o-tiles 4-6 + rowsum matmuls --------------
                for o in range(4, 7):
                    for bb in range(BB):
                        pos[(o, bb)] = pmain.tile([128, 512], F32,
                                                  tag="po", name="po")
                prs = []
                for k in range(KT):
                    for o in range(4, 7):
                        for bb in range(BB):
                            nc.tensor.matmul(pos[(o, bb)][:], w_blk(k, o),
                                             xk_slice(k, bb),
                                             start=(k == 0),
                                             stop=(k == KT - 1))
                    if k == 0:
                        # rowsum(x^2): ones-stationary broadcasts the
                        # column sum to every output partition. The aux
                        # banks recycle right after o3's flush.
                        for bb in range(BB):
                            p = paux.tile([128, 512], F32, tag="pa",
                                          name="prs")
                            nc.tensor.matmul(p[:], ones128[:],
                                             xsum[:, bb * 512:(bb + 1) * 512],
                                             start=True, stop=True)
                            prs.append(p)

                # u^T = sqrt(sigma^2 * rs + bsig^2[o]) straight out of PSUM
                s2 = float(sigma_const) * float(sigma_const)
                for o in range(OT):
                    ut = outs.tile([128, BS], BF, tag="u", bufs=3, name="ut")
                    for bb in range(BB):
                        bsl = slice(bb * 512, (bb + 1) * 512)
                        nc.scalar.activation(ut[:, bsl], prs[bb][:], AF.Sqrt,
                                             scale=s2,
                                             bias=bs2_all[:, o:o + 1])
                    nc.gpsimd.dma_start(u_d[o * 128:(o + 1) * 128, :], ut[:])

                for o in range(4, 7):
                    flush(o)

                # ---- phase 3: o-tile 7 ----------------------------------
                for bb in range(BB):
                    pos[(7, bb)] = pmain.tile([128, 512], F32,
                                              tag="po", name="po")
                for k in range(KT):
                    for bb in range(BB):
                        nc.tensor.matmul(pos[(7, bb)][:], w_blk(k, 7),
                                         xk_slice(k, bb),
                                         start=(k == 0),
                                         stop=(k == KT - 1))
                flush(7)
            else:
                # ---------------- general path ---------------------------
                nc.scalar.activation(bsig[:], bls_sb[:], AF.Exp)
                nc.vector.tensor_tensor(bse[:], bsig[:], beps_sb[:],
                                        ALU.mult)
                nc.vector.tensor_tensor(bias_all[:], bmu_sb[:], bse[:],
                                        ALU.add)
                nc.vector.tensor_tensor(bs2_all[:], bsig[:], bsig[:],
                                        ALU.mult)

                CHUNKS = [1, 1, 2, 4, 4, 4]       # ramped k-tile chunks
                k0g = 0
                for cn in CHUNKS:
                    xsl = slice(k0g * BS, (k0g + cn) * BS)
                    wsl = slice(k0g * OS, (k0g + cn) * OS)
                    nc.scalar.dma_start(xT[:, xsl], x_d[:, xsl])
                    mu_t = stage.tile([128, 4 * OS], BF, tag="mu", bufs=2)
                    nc.sync.dma_start(mu_t[:, :cn * OS], mu_d[:, wsl])
                    eps_t = stage.tile([128, 4 * OS], BF, tag="eps", bufs=2)
                    nc.scalar.dma_start(eps_t[:, :cn * OS], eps_d[:, wsl])
                    ls_t = stage.tile([128, 4 * OS], BF, tag="ls", bufs=2)
                    nc.sync.dma_start(ls_t[:, :cn * OS], ls_d[:, wsl])

                    for v0 in range(0, cn, 2):
                        vn = min(2, cn - v0)
                        lsl = slice(v0 * OS, (v0 + vn) * OS)
                        gsl = slice((k0g + v0) * OS, (k0g + v0 + vn) * OS)
                        xvsl = slice((k0g + v0) * BS, (k0g + v0 + vn) * BS)
                        se_t = stage.tile([128, 2 * OS], BF, tag="se",
                                          bufs=2)
                        sig_t = stage.tile([128, 2 * OS], BF, tag="sig",
                                           bufs=2)
                        nc.scalar.activation(sig_t[:, :vn * OS], ls_t[:, lsl],
                                             AF.Exp)
                        nc.vector.tensor_tensor(se_t[:, :vn * OS],
                                                sig_t[:, :vn * OS],
                                                eps_t[:, lsl], ALU.mult)
                        nc.vector.tensor_tensor(s2T[:, gsl],
                                                sig_t[:, :vn * OS],
                                                sig_t[:, :vn * OS], ALU.mult)
                        nc.vector.tensor_tensor(wT[:, gsl], mu_t[:, lsl],
                                                se_t[:, :vn * OS], ALU.add)
                        nc.scalar.activation(x2T[:, xvsl], xT[:, xvsl],
                                             AF.Square)
                    k0g += cn

                def w_blk(k, o):
                    c = (k * OT + o) * 128
                    return wT[:, c:c + 128]

                def s2_blk(k, o):
                    c = (k * OT + o) * 128
                    return s2T[:, c:c + 128]

                def x_sl(k, bb):
                    c = k * BS + bb * 512
                    return xT[:, c:c + 512]

                def x2_sl(k, bb):
                    c = k * BS + bb * 512
                    return x2T[:, c:c + 512]

                for o in range(OT):
                    ot = outs.tile([128, BS], BF, tag="o", bufs=3, name="ot")
                    ut = outs.tile([128, BS], BF, tag="u", bufs=3, name="ut")
                    for bb in range(BB):
                        bsl = slice(bb * 512, (bb + 1) * 512)
                        po = pmain.tile([128, 512], F32, tag="po", name="po")
                        for k in range(KT):
                            nc.tensor.matmul(po[:], w_blk(k, o), x_sl(k, bb),
                                             start=(k == 0),
                                             stop=(k == KT - 1))
                        nc.vector.tensor_scalar_add(ot[:, bsl], po[:],
                                                    bias_all[:, o:o + 1])
                        pu = pmain.tile([128, 512], F32, tag="po", name="pu")
                        for k in range(KT):
                            nc.tensor.matmul(pu[:], s2_blk(k, o),
                                             x2_sl(k, bb),
                                             start=(k == 0),
                                             stop=(k == KT - 1))
                        nc.scalar.activation(ut[:, bsl], pu[:], AF.Sqrt,
                                             bias=bs2_all[:, o:o + 1])
                    nc.sync.dma_start(o_d[o * 128:(o + 1) * 128, :], ot[:])
                    nc.scalar.dma_start(u_d[o * 128:(o + 1) * 128, :], ut[:])

    nc.compile()
    return nc


def _ktile_major(aT, width):
    """[IN, W] (contraction-major) -> [128, KT*W] with k-tile t at free
    cols [t*W, (t+1)*W)."""
    return np.ascontiguousarray(
        aT.reshape(KT, 128, width).transpose(1, 0, 2).reshape(128, KT * width))


def _weight_blocks(aT):
    """[IN, OS] -> [128, KT*OS] with contiguous 128-wide (k,o) blocks:
    free col = (k*OT + o)*128 + c."""
    return np.ascontiguousarray(
        aT.reshape(KT, 128, OT, 128).transpose(1, 0, 2, 3).reshape(
            128, KT * OS))


def _bias_grid(v):
    """[OS] fp32 slice -> [128, OT] grid with o-tile t in column t."""
    return np.ascontiguousarray(
        np.asarray(v, dtype=np.float32).reshape(OT, 128).T)


def kernel(x, weight_mu, weight_log_sigma, bias_mu, bias_log_sigma,
           eps_w, eps_b):
    global LAST_RESULT
    from concourse.bass_utils import run_bass_kernel_spmd

    x = np.asarray(x, dtype=np.float32)
    weight_mu = np.asarray(weight_mu, dtype=np.float32)
    weight_log_sigma = np.asarray(weight_log_sigma, dtype=np.float32)
    bias_mu = np.asarray(bias_mu, dtype=np.float32)
    bias_log_sigma = np.asarray(bias_log_sigma, dtype=np.float32)
    eps_w = np.asarray(eps_w, dtype=np.float32)
    eps_b = np.asarray(eps_b, dtype=np.float32)

    ls0 = weight_log_sigma.flat[0]
    fast = bool(np.all(weight_log_sigma == ls0))
    sigma_const = float(np.exp(np.float32(ls0))) if fast else None

    key = ("fast", sigma_const) if fast else ("general",)
    if key not in _compiled:
        _compiled[key] = _build(sigma_const)
    nc = _compiled[key]

    # host-side layout: transpose to contraction-major, downcast, tile
    xT = x.astype(BF16).T                                    # [IN, B] view
    muT = weight_mu.astype(BF16).T                           # [IN, OUT]
    epsT = eps_w.astype(FP8 if fast else BF16).T
    if not fast:
        lsT = weight_log_sigma.astype(BF16).T

    in_maps = []
    for i in range(R):
        for j in range(C):
            osl = slice(j * OS, (j + 1) * OS)
            m = {
                "x_sh": _ktile_major(
                    np.ascontiguousarray(xT[:, i * BS:(i + 1) * BS]), BS),
                "mu_sh": _weight_blocks(np.ascontiguousarray(muT[:, osl])),
                "eps_sh": _weight_blocks(np.ascontiguousarray(epsT[:, osl])),
                "bmu_sh": _bias_grid(bias_mu[osl]),
                "bls_sh": _bias_grid(bias_log_sigma[osl]),
                "beps_sh": _bias_grid(eps_b[osl]),
            }
            if not fast:
                m["ls_sh"] = _weight_blocks(np.ascontiguousarray(lsT[:, osl]))
            in_maps.append(m)

    res = run_bass_kernel_spmd(nc, in_maps, core_ids=list(range(N_CORES)),
                               trace=TRACE)
    LAST_RESULT = res

    output = np.empty((B, OUT), dtype=np.float32)
    uncertainty = np.empty((B, OUT), dtype=np.float32)
    for i in range(R):
        for j in range(C):
            c = i * C + j
            rsl = slice(i * BS, (i + 1) * BS)
            csl = slice(j * OS, (j + 1) * OS)
            output[rsl, csl] = res.results[c]["o_sh"].T.astype(np.float32)
            uncertainty[rsl, csl] = res.results[c]["u_sh"].T.astype(np.float32)
    return output, uncertainty
